# revision 19
# baseline (speedup 1.0000x reference)
"""Trainium2 Bass kernel for nn_Attention (dense transformer attention block).

Reference semantics (B=2, S=2048, D=2048, NH=16, NKV=4, HD=128):
    qkv = x @ wqkv.T ; split q/k/v ; rmsnorm(q), rmsnorm(k) (weights == 1)
    rotary(q), rotary(k) with arbitrary freqs_cis ; GQA repeat kv 4x
    causal softmax attention ; out = y @ wo.T

Sharding: 2-way data parallel over batch x 4-way tensor parallel over head
groups (each core owns 4 query heads + their single shared KV head).  Each
core computes a full-shape partial of the output projection for its batch
element; the host sums the 4 partials per batch element (the "all-reduce").

Device layout notes:
  - All tensors are kept "transposed" (feature dim on partitions, tokens on
    the free dim) so every matmul chains without transposes; only V is
    transposed on-device (PE transpose) to the [token, dv] layout the
    attention-value matmul needs as its stationary operand.
  - Head dims are permuted (even dims then odd dims) on the host so rotary
    becomes two contiguous 64-partition slabs; the permutation cancels in
    the q.k contraction and V/wo are left in natural order.
  - Softmax uses no running max: |scores| <= sqrt(128)*max|f|^2 is far below
    exp overflow in fp32 (verified empirically), so exp() is applied
    directly and the denominator is accumulated with a ones-vector matmul.
  - Rotary intermediates are bf16 so the vector engine runs in its 2x
    packed mode; output partials are stored bf16 (host accumulates fp32).
  - The out-projection is emitted as per-m-tile units woven into the NEXT
    query tile's attention stream: they are always-ready PE work placed at
    the points where the attention chain waits on exp, so the FIFO engine
    queue fills what would otherwise be stalls.
"""

import math
from contextlib import ExitStack

import numpy as np

B, S, D = 2, 2048, 2048
NH, NKV, HD = 16, 4, 128
EPS = 1e-6
N_CORES = 8
TPC = 4            # tensor-parallel cores per batch element
HEADS_PER_CORE = NH // TPC          # 4
Q_SIZE, KV_SIZE = NH * HD, NKV * HD
E_LOC = HEADS_PER_CORE * HD         # 512 local y/e dims per core
TT = 512                            # token tile (free dim) for matmuls
N_TT = S // TT                      # 4
N_KT = D // 128                     # 16 contraction tiles for projections
N_SKT = S // 128                    # 16 key tiles per sequence

_F32 = "float32"


def _steer_act_tables():
    """Make Exp and Ln both resolve to the combined natural_log_exp table.

    bacc's insert_act_table_loads picks the first act-function set that
    contains each function, which puts Exp and Ln in different tables and
    costs a ~1.3us ACT table re-load on every rmsnorm <-> softmax switch.
    Stripping Exp/Ln from the other sets (list positions preserved, so set
    ids stay valid for walrus) leaves one shared table and a single load.
    """
    from concourse import bacc
    import concourse.mybir as mybir
    import concourse.hw_specs as hw_specs

    if getattr(bacc.get_activation_tables, "_act_steered", False):
        return
    orig = hw_specs.get_activation_tables

    def steered(arch):
        tabs = orig(arch)
        for name, fns in tabs.items():
            if name != "natural_log_exp_and_others":
                fns.discard(mybir.ActivationFunctionType.Exp)
                fns.discard(mybir.ActivationFunctionType.Ln)
        return tabs

    steered._act_steered = True
    bacc.get_activation_tables = steered


def _build_bass():
    import concourse.bass as bass  # noqa: F401
    import concourse.mybir as mybir
    import concourse.tile as tile
    from concourse import bacc
    from concourse.masks import make_identity

    _steer_act_tables()

    f32 = mybir.dt.float32
    f16 = mybir.dt.float16
    bf16 = mybir.dt.bfloat16

    nc = bacc.Bacc("TRN2", target_bir_lowering=False, debug=False,
                   num_devices=N_CORES)

    # ---- DRAM I/O (per-core shards supplied via in_maps) ----
    xT_d = nc.dram_tensor("xT", (D, S), bf16, kind="ExternalInput").ap()
    # per-chunk-contiguous weight layout: [chunk, p, ko, e] so each chunk's
    # stationary tiles stream in with 4 KiB/partition contiguous lines
    wqkvT_d = nc.dram_tensor(
        "wqkvT", (HEADS_PER_CORE + 2, 128, N_KT, HD), bf16,
        kind="ExternalInput").ap()
    woT_d = nc.dram_tensor("woT", (E_LOC, D), bf16, kind="ExternalInput").ap()
    # fr/fi are duplicated across both 64-partition halves so rotary ops can
    # pair them with either the even (base 0) or odd (base 64) slab of q/k
    fr_d = nc.dram_tensor("fr", (HD, S), f32, kind="ExternalInput").ap()
    fi_d = nc.dram_tensor("fi", (HD, S), f32, kind="ExternalInput").ap()
    mask_d = nc.dram_tensor("mask", (128, TT), bf16,
                            kind="ExternalInput").ap()
    outT_d = nc.dram_tensor("outT", (D, S), bf16, kind="ExternalOutput").ap()

    NCHUNK = HEADS_PER_CORE + 2     # 4 q heads, 1 k head, 1 v head
    SCALE = 1.0 / math.sqrt(HD)

    with tile.TileContext(nc) as tc, ExitStack() as ctx:
        # ---------- pools ----------
        const = ctx.enter_context(tc.tile_pool(name="const", bufs=1))
        sb = ctx.enter_context(tc.tile_pool(name="sb", bufs=2))
        # output staging is deep: each slot is held through its DMA's ~2us
        # HBM completion receipt, and the out-proj tail is evacuation-paced
        osbp = ctx.enter_context(tc.tile_pool(name="osbp", bufs=6))
        epool = ctx.enter_context(tc.tile_pool(name="epool", bufs=4))
        prow = ctx.enter_context(tc.tile_pool(name="prow", bufs=1,
                                              space="PSUM"))

        # ---------- resident tensors ----------
        # phase-1-only tensors live in their own pool, freed before attention
        # needs peak SBUF
        p1_ctx = ExitStack()
        p1 = p1_ctx.enter_context(tc.tile_pool(name="p1", bufs=1))
        p1w = p1_ctx.enter_context(tc.tile_pool(name="p1w", bufs=3))
        pproj = p1_ctx.enter_context(tc.tile_pool(name="pproj", bufs=4,
                                                  space="PSUM"))
        ptp = p1_ctx.enter_context(tc.tile_pool(name="ptp", bufs=2,
                                                space="PSUM"))
        # DMA priority: the k-chunk weights + first xT tiles gate the first
        # matmul, so they go first; bulk/constant loads go on the gpsimd
        # DMA queue so their issue cost doesn't delay the critical loads.
        xT = p1.tile([128, N_KT, S], bf16)               # 64 KiB/part
        xT_r = xT_d.rearrange("(ko p) t -> p ko t", p=128)
        fr = p1.tile([HD, S], f32)
        fi = p1.tile([HD, S], f32)
        woT = const.tile([128, HEADS_PER_CORE, D], bf16)
        cmask = const.tile([128, TT], bf16)

        def load_wch(chunk):
            wch = p1w.tile([128, N_KT, HD], bf16, tag="wch", name="wch")
            nc.sync.dma_start(wch[:], wqkvT_d[chunk])
            return wch

        wch_next = load_wch(HEADS_PER_CORE)    # k-chunk weights first
        for half in range(2):
            hs = slice(half * (S // 2), (half + 1) * (S // 2))
            for kt in range(N_KT):
                # stream x in the order the projection consumes it: all k
                # tiles of the first token half, then the second half
                nc.sync.dma_start(xT[:, kt, hs], xT_r[:, kt, hs])
        nc.gpsimd.dma_start(fr[:], fr_d)
        nc.gpsimd.dma_start(fi[:], fi_d)
        nc.gpsimd.dma_start(cmask[:], mask_d)

        ident = const.tile([128, 128], bf16)
        make_identity(nc, ident[:])
        ones = const.tile([128, 1], bf16)
        nc.vector.memset(ones[:], 1.0)
        epsb = const.tile([1, 1], f32)
        nc.vector.memset(epsb[:], EPS)

        # rotated q (4 heads), rotated k, and v in [token, dv] layout
        qrot = [const.tile([128, S], bf16, tag=f"qrot{h}", name=f"qrot{h}")
                for h in range(HEADS_PER_CORE)]
        krot = const.tile([128, S], bf16)
        vT = const.tile([128, S], bf16)
        vtok = const.tile([128, N_SKT, HD], bf16)
        # normalized attention outputs (yT), stationary input of out-proj
        yT = [const.tile([128, S], bf16, tag=f"yT{h}", name=f"yT{h}")
              for h in range(HEADS_PER_CORE)]

        # ---------- phase 1 helpers: projection chunks (+norm+rotary) ------
        def proj_consume(chunk, tt, ps):
            is_v = chunk == HEADS_PER_CORE + 1
            is_k = chunk == HEADS_PER_CORE
            ts = slice(tt * TT, (tt + 1) * TT)
            if is_v:
                nc.vector.tensor_copy(vT[:, ts], ps[:])
            else:
                # rms stats: mean over head dim (partitions) via ones-matmul;
                # square runs on ACT (same table set as exp/ln)
                sq = sb.tile([128, TT], bf16, tag="sq", name="sq")
                nc.scalar.activation(sq[:], ps[:],
                                     mybir.ActivationFunctionType.Square)
                ms = prow.tile([1, TT], f32, tag="row", name="ms")
                nc.tensor.matmul(ms[:], ones[:], sq[:], start=True, stop=True)
                lnms = sb.tile([1, TT], f32, tag="lnms", name="lnms")
                nc.scalar.activation(lnms[:], ms[:],
                                     mybir.ActivationFunctionType.Ln,
                                     bias=epsb[:], scale=1.0 / HD)
                rs = sb.tile([1, TT], f32, tag="rs", name="rs")
                nc.scalar.activation(rs[:], lnms[:],
                                     mybir.ActivationFunctionType.Exp,
                                     bias=0.0, scale=-0.5)
                rsb = sb.tile([128, TT], f32, tag="rsb", name="rsb")
                nc.gpsimd.partition_broadcast(rsb[:], rs[:])
                # rotary, even dims on partitions 0:64, odd on 64:128:
                #   a      = q * fr            (both halves at once)
                #   bswap  = swap_halves(q) * [+fi; -fi]  (2 cross-half muls;
                #            the sign baked into fi makes the combine an add)
                #   rot    = a + bswap
                # kept fp32 until the final bf16 store: the rounding error
                # of bf16 intermediates is amplified ~|score| by the exp
                rot = sb.tile([128, TT], f32, tag="rot", name="rot")
                a = sb.tile([128, TT], f32, tag="rota", name="a")
                nc.vector.tensor_mul(a[:], ps[:], fr[:, ts])
                bsw = sb.tile([128, TT], f32, tag="rotb", name="bsw")
                nc.vector.tensor_mul(bsw[0:64, :], ps[64:128, :],
                                     fi[64:128, ts])
                nc.vector.tensor_mul(bsw[64:128, :], ps[0:64, :],
                                     fi[0:64, ts])
                nc.vector.tensor_add(rot[:], a[:], bsw[:])
                dst = krot if is_k else qrot[chunk]
                nc.vector.tensor_mul(dst[:, ts], rot[:], rsb[:])

        def project_chunk(chunk, wch=None):
            if wch is None:
                wch = load_wch(chunk)
            for tt in range(N_TT):
                ts = slice(tt * TT, (tt + 1) * TT)
                ps = pproj.tile([128, TT], f32, tag="proj", name="ps")
                for kt in range(N_KT):
                    nc.tensor.matmul(
                        ps[:], wch[:, kt, :],
                        xT[:, kt, ts], start=(kt == 0), stop=(kt == N_KT - 1))
                proj_consume(chunk, tt, ps)

        # ---------- emission: projections ----------
        # The k chunk runs while x is still streaming in: its two tiles per
        # token half are interleaved with a 6-kt phase lag so the lagging
        # tile's matmuls (whose x pieces arrived long ago) fill the DMA
        # arrival gaps that otherwise idle the PE during the ramp, while
        # the tiles still finish ~4us apart so their rms/rotary consumer
        # chains pipeline instead of colliding.
        LAGK = 6
        for pair in ((0, 1), (2, 3)):
            pss = [pproj.tile([128, TT], f32, tag="proj", name="ps")
                   for _ in pair]
            tss = [slice(tt * TT, (tt + 1) * TT) for tt in pair]
            for r in range(N_KT + LAGK):
                if r < N_KT:
                    nc.tensor.matmul(
                        pss[0][:], wch_next[:, r, :], xT[:, r, tss[0]],
                        start=(r == 0), stop=(r == N_KT - 1))
                if r >= LAGK:
                    kt = r - LAGK
                    nc.tensor.matmul(
                        pss[1][:], wch_next[:, kt, :], xT[:, kt, tss[1]],
                        start=(kt == 0), stop=(kt == N_KT - 1))
            for tt, ps in zip(pair, pss):
                proj_consume(HEADS_PER_CORE, tt, ps)
        project_chunk(HEADS_PER_CORE + 1)              # v
        for h in range(HEADS_PER_CORE):
            project_chunk(h)
        # v -> [token, dv] layout; emitted last so these ready-to-run PE ops
        # fill the gap while the final q chunk's rotary drains the PSUM
        # banks the attention pools alias
        for i in range(N_SKT):
            tp = ptp.tile([128, 128], bf16, tag="tp", name="tp")
            nc.tensor.transpose(tp[:], vT[:, i * 128:(i + 1) * 128], ident[:])
            nc.vector.tensor_copy(vtok[:, i, :], tp[:])
        # wo stream goes last on the sync queue: it is only needed by the
        # first out-projection (~40us later) and must not steal HBM
        # bandwidth from the x/wqkv stream that gates phase 1.
        nc.sync.dma_start(
            woT[:], woT_d.rearrange("(eo p) d -> p eo d", p=128))
        p1_ctx.close()   # xT/wqkvT/fr/fi + projection PSUM no longer needed

        # ---------- phase 2 pools (reuse the PSUM banks phase 1 freed) ----
        psum = ctx.enter_context(tc.tile_pool(name="psum", bufs=2,
                                              space="PSUM"))
        pacc = ctx.enter_context(tc.tile_pool(name="pacc", bufs=1,
                                              space="PSUM"))
        pout = ctx.enter_context(tc.tile_pool(name="pout", bufs=2,
                                              space="PSUM"))

        # ---------- out-projection m-tile unit (PE filler work) ----------
        # evacuation alternates DVE/ACT so consecutive m-tiles' PSUM-bank
        # releases overlap (the dense tail is otherwise evacuation-paced)
        evac_flip = [0]

        def outproj_mtile(qt, m):
            qs = slice(qt * TT, (qt + 1) * TT)
            ops = pout.tile([128, TT], f32, tag="oproj", name="ops")
            for e in range(HEADS_PER_CORE):
                nc.tensor.matmul(ops[:],
                                 woT[:, e, m * 128:(m + 1) * 128],
                                 yT[e][:, qs], start=(e == 0),
                                 stop=(e == HEADS_PER_CORE - 1))
            osb = osbp.tile([128, TT], bf16, tag="osb", name="osb")
            if evac_flip[0] % 2:
                nc.scalar.copy(osb[:], ops[:])
            else:
                nc.vector.tensor_copy(osb[:], ops[:])
            evac_flip[0] += 1
            nc.sync.dma_start(outT_d[m * 128:(m + 1) * 128, qs], osb[:])

        pending = []     # out-proj m-tiles ready to weave into the PE stream

        def weave_one():
            if pending:
                outproj_mtile(*pending.pop(0))

        # ---------- attention unit (head h, query tile qt) ----------
        # Two key-tiles of scores share one 2-bank fp32 PSUM tile and one
        # exp instruction (halves the per-exp overhead), and the denom/y
        # consumer matmuls are software-pipelined LAG pairs behind the
        # score matmuls: the exp latency is then never on the PE critical
        # path, and yps/dps single-buffer without boundary stalls (a score
        # PSUM tile frees at its exp, not at its consumers).
        LAG = 2

        def attention_unit(h, qt, weave=0):
            ntk = 4 * (qt + 1)
            npair = ntk // 2
            dps = prow.tile([1, TT], f32, tag="row", name="dps")
            yps = pacc.tile([128, TT], f32, tag="yacc", name="yps")
            stages = []    # per-pair consumer args: (e, halves)

            def emit_scores(p):
                sps = psum.tile([128, 2, TT], f32, tag="mm", name="sps")
                halves = []
                for i in (0, 1):
                    # diagonal tiles (r >= 1) only have valid scores in
                    # their last TT - 128*r columns; skip the fully-masked
                    # prefix.  In suffix-local coords the causal mask is
                    # always the r=0 triangle.
                    tk = 2 * p + i
                    r = tk - 4 * qt
                    off = 128 * r if r > 0 else 0
                    qs = slice(qt * TT + off, (qt + 1) * TT)
                    nc.tensor.matmul(sps[:, i, off:],
                                     krot[:, tk * 128:(tk + 1) * 128],
                                     qrot[h][:, qs], start=True, stop=True)
                    halves.append((tk, r, off, TT - off))
                e = epool.tile([128, 2, TT], bf16, tag="e", name="e")
                if halves[0][1] < 0 and halves[1][1] < 0:
                    nc.scalar.activation(e[:], sps[:],
                                         mybir.ActivationFunctionType.Exp,
                                         bias=0.0, scale=SCALE)
                else:
                    for i, (tk, r, off, w) in enumerate(halves):
                        nc.scalar.activation(
                            e[:, i, off:], sps[:, i, off:],
                            mybir.ActivationFunctionType.Exp,
                            bias=0.0, scale=SCALE)
                stages.append((e, halves))

            def emit_consumers(p):
                e, halves = stages[p]
                full = halves[0][1] < 0 and halves[1][1] < 0
                if full:
                    # one denominator matmul per pair on the DVE pair-sum
                    # (single bf16 rounding of the pair, not a running sum)
                    e01 = epool.tile([128, TT], bf16, tag="em", name="e01")
                    nc.vector.tensor_add(e01[:], e[:, 0, :], e[:, 1, :])
                    nc.tensor.matmul(dps[:], ones[:], e01[:],
                                     start=(halves[0][0] == 0),
                                     stop=(halves[1][0] == ntk - 1))
                for i, (tk, r, off, w) in enumerate(halves):
                    src = e[:, i, off:]
                    if r >= 0:
                        em = epool.tile([128, TT], bf16, tag="em", name="em")
                        nc.vector.tensor_mul(em[:, :w], e[:, i, off:],
                                             cmask[:, :w])
                        src = em[:, :w]
                        nc.tensor.matmul(dps[:, off:], ones[:], src,
                                         start=(tk == 0),
                                         stop=(tk == ntk - 1))
                    elif not full:
                        nc.tensor.matmul(dps[:, off:], ones[:], src,
                                         start=(tk == 0),
                                         stop=(tk == ntk - 1))
                    nc.tensor.matmul(yps[:, off:], vtok[:, tk, :], src,
                                     start=(tk == 0), stop=(tk == ntk - 1))

            for p in range(npair):
                emit_scores(p)
                if p >= LAG:
                    emit_consumers(p - LAG)
                # drop an always-ready out-proj unit into the PE queue so
                # the engine has work while the next exp drains
                for _ in range(weave):
                    weave_one()
            for p in range(max(0, npair - LAG), npair):
                emit_consumers(p)
            qs = slice(qt * TT, (qt + 1) * TT)
            dr = sb.tile([1, TT], f32, tag="dr", name="dr")
            nc.vector.reciprocal_approx_fast(dr[:], dps[:])
            drb = sb.tile([128, TT], f32, tag="drb", name="drb")
            nc.gpsimd.partition_broadcast(drb[:], dr[:])
            nc.vector.tensor_mul(yT[h][:, qs], yps[:], drb[:])

        # ---------- emission: attention qt=3..0 with woven out-proj -------
        # out-proj for query tile qt becomes available once all 4 heads of
        # qt are done; it is woven into the following qt's attention.
        for qt in (3, 2, 1, 0):
            # more weave slots as attention units shrink
            weave = {3: 0, 2: 1, 1: 1, 0: 2}[qt]
            for h in range(HEADS_PER_CORE):
                attention_unit(h, qt, weave=weave)
            pending.extend((qt, m) for m in range(D // 128))
        while pending:
            weave_one()

    nc.compile()
    return nc


def _host_shards(x, freqs_cis, wqkv, wo):
    import ml_dtypes
    bf16 = ml_dtypes.bfloat16

    # head-dim permutation: even dims then odd dims (for q and k only)
    perm = np.concatenate([np.arange(0, HD, 2), np.arange(1, HD, 2)])

    wq = wqkv[:Q_SIZE].reshape(NH, HD, D)[:, perm, :]
    wk = wqkv[Q_SIZE:Q_SIZE + KV_SIZE].reshape(NKV, HD, D)[:, perm, :]
    wv = wqkv[Q_SIZE + KV_SIZE:].reshape(NKV, HD, D)

    fr1 = np.ascontiguousarray(freqs_cis[:, :, 0].T, dtype=np.float32)
    fi1 = np.ascontiguousarray(freqs_cis[:, :, 1].T, dtype=np.float32)
    fr = np.vstack([fr1, fr1])
    # sign baked in so the rotary combine is a single add:
    #   rot[lo] = q_lo*fr + q_hi*(-fi) ; rot[hi] = q_hi*fr + q_lo*(+fi)
    fi = np.vstack([fi1, -fi1])

    # causal mask for the leading diagonal of a 128-row x 512-col score
    # tile (suffix-narrowed diagonal tiles reuse its prefix columns)
    tkl = np.arange(128)[:, None]
    tql = np.arange(TT)[None, :]
    mask = (tkl <= tql).astype(bf16)

    in_maps = []
    for c in range(N_CORES):
        b, j = divmod(c, TPC)
        wshard = np.concatenate(
            [wq[TPC * j + h] for h in range(HEADS_PER_CORE)] +
            [wk[j], wv[j]], axis=0)                     # (768, D)
        # [chunk, p, ko, e] with d = ko*128 + p
        wpack = np.ascontiguousarray(
            wshard.reshape(HEADS_PER_CORE + 2, HD, N_KT, 128)
            .transpose(0, 3, 2, 1)).astype(bf16)
        in_maps.append({
            "xT": np.ascontiguousarray(x[b].T).astype(bf16),
            "wqkvT": wpack,
            "woT": np.ascontiguousarray(
                wo[:, j * E_LOC:(j + 1) * E_LOC].T).astype(bf16),
            "fr": fr,
            "fi": fi,
            "mask": mask,
        })
    return in_maps


_NC_CACHE = {}


def _get_nc():
    if "nc" not in _NC_CACHE:
        _NC_CACHE["nc"] = _build_bass()
    return _NC_CACHE["nc"]


def kernel(x, freqs_cis, wqkv, wo, q_norm_w, k_norm_w, _want_results=False):
    # q_norm_w / k_norm_w are all-ones per the problem spec; rmsnorm weight
    # multiply is the identity and is folded away.
    from concourse.bass_utils import run_bass_kernel_spmd

    nc = _get_nc()
    in_maps = _host_shards(np.asarray(x, np.float32),
                           np.asarray(freqs_cis, np.float32),
                           np.asarray(wqkv, np.float32),
                           np.asarray(wo, np.float32))
    res = run_bass_kernel_spmd(nc, in_maps, core_ids=list(range(N_CORES)))
    parts = [r["outT"] for r in res.results]
    out = np.empty((B, S, D), np.float32)
    for b in range(B):
        acc = parts[TPC * b].astype(np.float32)
        for j in range(1, TPC):
            acc = acc + parts[TPC * b + j].astype(np.float32)
        out[b] = acc.T
    if _want_results:
        return out, res
    return out


# revision 22
# speedup vs baseline: 1.1603x; 1.1603x over previous
"""Trainium2 Bass kernel for nn_Attention (dense transformer attention block).

Reference semantics (B=2, S=2048, D=2048, NH=16, NKV=4, HD=128):
    qkv = x @ wqkv.T ; split q/k/v ; rmsnorm(q), rmsnorm(k) (weights == 1)
    rotary(q), rotary(k) with arbitrary freqs_cis ; GQA repeat kv 4x
    causal softmax attention ; out = y @ wo.T

Sharding: 2-way data parallel over batch x 4-way tensor parallel over head
groups (each core owns 4 query heads + their single shared KV head).  Each
core computes a full-shape partial of the output projection for its batch
element; the host sums the 4 partials per batch element (the "all-reduce").

Device layout notes:
  - All tensors are kept "transposed" (feature dim on partitions, tokens on
    the free dim) so every matmul chains without transposes; only V is
    transposed on-device (PE transpose) to the [token, dv] layout the
    attention-value matmul needs as its stationary operand.
  - Head dims are permuted (even dims then odd dims) on the host so rotary
    becomes two contiguous 64-partition slabs; the permutation cancels in
    the q.k contraction and V/wo are left in natural order.
  - Softmax uses no running max: |scores| <= sqrt(128)*max|f|^2 is far below
    exp overflow in fp32 (verified empirically), so exp() is applied
    directly and the denominator is accumulated with a ones-vector matmul.
  - Rotary intermediates are bf16 so the vector engine runs in its 2x
    packed mode; output partials are stored bf16 (host accumulates fp32).
  - The out-projection is emitted as per-m-tile units woven into the NEXT
    query tile's attention stream: they are always-ready PE work placed at
    the points where the attention chain waits on exp, so the FIFO engine
    queue fills what would otherwise be stalls.
"""

import math
from contextlib import ExitStack

import numpy as np

B, S, D = 2, 2048, 2048
NH, NKV, HD = 16, 4, 128
EPS = 1e-6
N_CORES = 8
TPC = 4            # tensor-parallel cores per batch element
HEADS_PER_CORE = NH // TPC          # 4
Q_SIZE, KV_SIZE = NH * HD, NKV * HD
E_LOC = HEADS_PER_CORE * HD         # 512 local y/e dims per core
TT = 512                            # token tile (free dim) for matmuls
N_TT = S // TT                      # 4
N_KT = D // 128                     # 16 contraction tiles for projections
N_SKT = S // 128                    # 16 key tiles per sequence

_F32 = "float32"


def _steer_act_tables():
    """Make Exp and Ln both resolve to the combined natural_log_exp table.

    bacc's insert_act_table_loads picks the first act-function set that
    contains each function, which puts Exp and Ln in different tables and
    costs a ~1.3us ACT table re-load on every rmsnorm <-> softmax switch.
    Stripping Exp/Ln from the other sets (list positions preserved, so set
    ids stay valid for walrus) leaves one shared table and a single load.
    """
    from concourse import bacc
    import concourse.mybir as mybir
    import concourse.hw_specs as hw_specs

    if getattr(bacc.get_activation_tables, "_act_steered", False):
        return
    orig = hw_specs.get_activation_tables

    def steered(arch):
        tabs = orig(arch)
        for name, fns in tabs.items():
            if name != "natural_log_exp_and_others":
                fns.discard(mybir.ActivationFunctionType.Exp)
                fns.discard(mybir.ActivationFunctionType.Ln)
        return tabs

    steered._act_steered = True
    bacc.get_activation_tables = steered


def _build_bass():
    import concourse.bass as bass  # noqa: F401
    import concourse.mybir as mybir
    import concourse.tile as tile
    from concourse import bacc
    from concourse.masks import make_identity

    _steer_act_tables()

    f32 = mybir.dt.float32
    f16 = mybir.dt.float16
    bf16 = mybir.dt.bfloat16

    nc = bacc.Bacc("TRN2", target_bir_lowering=False, debug=False,
                   num_devices=N_CORES)

    # ---- DRAM I/O (per-core shards supplied via in_maps) ----
    xT_d = nc.dram_tensor("xT", (D, S), bf16, kind="ExternalInput").ap()
    # per-chunk-contiguous weight layout: [chunk, p, ko, e] so each chunk's
    # stationary tiles stream in with 4 KiB/partition contiguous lines
    wqkvT_d = nc.dram_tensor(
        "wqkvT", (HEADS_PER_CORE + 2, 128, N_KT, HD), bf16,
        kind="ExternalInput").ap()
    woT_d = nc.dram_tensor("woT", (E_LOC, D), bf16, kind="ExternalInput").ap()
    # fr/fi are duplicated across both 64-partition halves so rotary ops can
    # pair them with either the even (base 0) or odd (base 64) slab of q/k
    fr_d = nc.dram_tensor("fr", (HD, S), f32, kind="ExternalInput").ap()
    fi_d = nc.dram_tensor("fi", (HD, S), f32, kind="ExternalInput").ap()
    mask_d = nc.dram_tensor("mask", (128, TT), bf16,
                            kind="ExternalInput").ap()
    outT_d = nc.dram_tensor("outT", (D, S), bf16, kind="ExternalOutput").ap()

    NCHUNK = HEADS_PER_CORE + 2     # 4 q heads, 1 k head, 1 v head
    SCALE = 1.0 / math.sqrt(HD)

    with tile.TileContext(nc) as tc, ExitStack() as ctx:
        # ---------- pools ----------
        const = ctx.enter_context(tc.tile_pool(name="const", bufs=1))
        sb = ctx.enter_context(tc.tile_pool(name="sb", bufs=2))
        # output staging is deep: each slot is held through its DMA's ~2us
        # HBM completion receipt, and the out-proj tail is evacuation-paced
        osbp = ctx.enter_context(tc.tile_pool(name="osbp", bufs=6))
        epool = ctx.enter_context(tc.tile_pool(name="epool", bufs=4))
        prow = ctx.enter_context(tc.tile_pool(name="prow", bufs=1,
                                              space="PSUM"))

        # ---------- resident tensors ----------
        # phase-1-only tensors live in their own pool, freed before attention
        # needs peak SBUF
        p1_ctx = ExitStack()
        p1 = p1_ctx.enter_context(tc.tile_pool(name="p1", bufs=1))
        p1w = p1_ctx.enter_context(tc.tile_pool(name="p1w", bufs=3))
        pproj = p1_ctx.enter_context(tc.tile_pool(name="pproj", bufs=4,
                                                  space="PSUM"))
        ptp = p1_ctx.enter_context(tc.tile_pool(name="ptp", bufs=2,
                                                space="PSUM"))
        # DMA priority: the k-chunk weights + first xT tiles gate the first
        # matmul, so they go first; bulk/constant loads go on the gpsimd
        # DMA queue so their issue cost doesn't delay the critical loads.
        xT = p1.tile([128, N_KT, S], bf16)               # 64 KiB/part
        xT_r = xT_d.rearrange("(ko p) t -> p ko t", p=128)
        fr = p1.tile([HD, S], f32)
        fi = p1.tile([HD, S], f32)
        woT = const.tile([128, HEADS_PER_CORE, D], bf16)
        cmask = const.tile([128, TT], bf16)

        def load_wch(chunk):
            wch = p1w.tile([128, N_KT, HD], bf16, tag="wch", name="wch")
            nc.sync.dma_start(wch[:], wqkvT_d[chunk])
            return wch

        wch_next = load_wch(HEADS_PER_CORE)    # k-chunk weights first
        wch_v = None
        for half in range(2):
            hs = slice(half * (S // 2), (half + 1) * (S // 2))
            for kt in range(N_KT):
                # stream x in the order the projection consumes it: all k
                # tiles of the first token half, then the second half
                nc.sync.dma_start(xT[:, kt, hs], xT_r[:, kt, hs])
            if half == 0:
                # v weights slot between the x halves: ready right when the
                # v tiles join the ramp group that fills half-1 DMA gaps
                wch_v = load_wch(HEADS_PER_CORE + 1)
        nc.gpsimd.dma_start(fr[:], fr_d)
        nc.gpsimd.dma_start(fi[:], fi_d)
        nc.gpsimd.dma_start(cmask[:], mask_d)

        ident = const.tile([128, 128], bf16)
        make_identity(nc, ident[:])
        ones = const.tile([128, 1], bf16)
        nc.vector.memset(ones[:], 1.0)
        epsb = const.tile([1, 1], f32)
        nc.vector.memset(epsb[:], EPS)

        # rotated q (4 heads), rotated k, and v in [token, dv] layout
        qrot = [const.tile([128, S], bf16, tag=f"qrot{h}", name=f"qrot{h}")
                for h in range(HEADS_PER_CORE)]
        krot = const.tile([128, S], bf16)
        vT = const.tile([128, S], bf16)
        vtok = const.tile([128, N_SKT, HD], bf16)
        # normalized attention outputs (yT), stationary input of out-proj
        yT = [const.tile([128, S], bf16, tag=f"yT{h}", name=f"yT{h}")
              for h in range(HEADS_PER_CORE)]

        # ---------- phase 1 helpers: projection chunks (+norm+rotary) ------
        def proj_consume(chunk, tt, ps):
            is_v = chunk == HEADS_PER_CORE + 1
            is_k = chunk == HEADS_PER_CORE
            ts = slice(tt * TT, (tt + 1) * TT)
            if is_v:
                nc.vector.tensor_copy(vT[:, ts], ps[:])
            else:
                # rms stats: mean over head dim (partitions) via ones-matmul;
                # square runs on ACT (same table set as exp/ln)
                sq = sb.tile([128, TT], bf16, tag="sq", name="sq")
                nc.scalar.activation(sq[:], ps[:],
                                     mybir.ActivationFunctionType.Square)
                ms = prow.tile([1, TT], f32, tag="row", name="ms")
                nc.tensor.matmul(ms[:], ones[:], sq[:], start=True, stop=True)
                lnms = sb.tile([1, TT], f32, tag="lnms", name="lnms")
                nc.scalar.activation(lnms[:], ms[:],
                                     mybir.ActivationFunctionType.Ln,
                                     bias=epsb[:], scale=1.0 / HD)
                rs = sb.tile([1, TT], f32, tag="rs", name="rs")
                nc.scalar.activation(rs[:], lnms[:],
                                     mybir.ActivationFunctionType.Exp,
                                     bias=0.0, scale=-0.5)
                rsb = sb.tile([128, TT], f32, tag="rsb", name="rsb")
                nc.gpsimd.partition_broadcast(rsb[:], rs[:])
                # rotary, even dims on partitions 0:64, odd on 64:128:
                #   a      = q * fr            (both halves at once)
                #   bswap  = swap_halves(q) * [+fi; -fi]  (2 cross-half muls;
                #            the sign baked into fi makes the combine an add)
                #   rot    = a + bswap
                # kept fp32 until the final bf16 store: the rounding error
                # of bf16 intermediates is amplified ~|score| by the exp
                rot = sb.tile([128, TT], f32, tag="rot", name="rot")
                a = sb.tile([128, TT], f32, tag="rota", name="a")
                nc.vector.tensor_mul(a[:], ps[:], fr[:, ts])
                bsw = sb.tile([128, TT], f32, tag="rotb", name="bsw")
                nc.vector.tensor_mul(bsw[0:64, :], ps[64:128, :],
                                     fi[64:128, ts])
                nc.vector.tensor_mul(bsw[64:128, :], ps[0:64, :],
                                     fi[0:64, ts])
                nc.vector.tensor_add(rot[:], a[:], bsw[:])
                dst = krot if is_k else qrot[chunk]
                nc.vector.tensor_mul(dst[:, ts], rot[:], rsb[:])

        def project_chunk(chunk, wch=None):
            if wch is None:
                wch = load_wch(chunk)
            for tt in range(N_TT):
                ts = slice(tt * TT, (tt + 1) * TT)
                ps = pproj.tile([128, TT], f32, tag="proj", name="ps")
                for kt in range(N_KT):
                    nc.tensor.matmul(
                        ps[:], wch[:, kt, :],
                        xT[:, kt, ts], start=(kt == 0), stop=(kt == N_KT - 1))
                proj_consume(chunk, tt, ps)

        # ---------- emission: projections ----------
        # The k chunk runs while x is still streaming in: tiles are
        # interleaved with a phase lag so the lagging tiles' matmuls
        # (whose x pieces arrived long ago) fill the DMA arrival gaps
        # that otherwise idle the PE during the ramp, and the tiles
        # finish staggered so their rms/rotary consumer chains pipeline.
        # The half-1 ramp group adds the two half-0 v tiles (x resident,
        # weights preloaded) as always-ready filler.
        LAGK = 3
        KCH, VCH = HEADS_PER_CORE, HEADS_PER_CORE + 1

        def ramp_group(plan):
            # plan: list of (chunk, wch, tt, lag)
            pss, tss = [], []
            for chunk, wch, tt, lag in plan:
                pss.append(pproj.tile([128, TT], f32, tag="proj", name="ps"))
                tss.append(slice(tt * TT, (tt + 1) * TT))
            maxlag = max(lag for _, _, _, lag in plan)
            for r in range(N_KT + maxlag):
                for (chunk, wch, tt, lag), ps, ts in zip(plan, pss, tss):
                    kt = r - lag
                    if 0 <= kt < N_KT:
                        nc.tensor.matmul(
                            ps[:], wch[:, kt, :], xT[:, kt, ts],
                            start=(kt == 0), stop=(kt == N_KT - 1))
            for (chunk, wch, tt, lag), ps in zip(plan, pss):
                proj_consume(chunk, tt, ps)

        ramp_group([(KCH, wch_next, 0, 0), (KCH, wch_next, 1, LAGK)])
        ramp_group([(KCH, wch_next, 2, 0), (KCH, wch_next, 3, LAGK),
                    (VCH, wch_v, 0, 0), (VCH, wch_v, 1, 0)])
        for tt in (2, 3):
            ts = slice(tt * TT, (tt + 1) * TT)
            ps = pproj.tile([128, TT], f32, tag="proj", name="ps")
            for kt in range(N_KT):
                nc.tensor.matmul(ps[:], wch_v[:, kt, :], xT[:, kt, ts],
                                 start=(kt == 0), stop=(kt == N_KT - 1))
            proj_consume(VCH, tt, ps)
        for h in range(HEADS_PER_CORE):
            project_chunk(h)
        # v -> [token, dv] layout; emitted last so these ready-to-run PE ops
        # fill the gap while the final q chunk's rotary drains the PSUM
        # banks the attention pools alias
        for i in range(N_SKT):
            tp = ptp.tile([128, 128], bf16, tag="tp", name="tp")
            nc.tensor.transpose(tp[:], vT[:, i * 128:(i + 1) * 128], ident[:])
            nc.vector.tensor_copy(vtok[:, i, :], tp[:])
        # wo stream goes last on the sync queue: it is only needed by the
        # first out-projection (~40us later) and must not steal HBM
        # bandwidth from the x/wqkv stream that gates phase 1.
        nc.sync.dma_start(
            woT[:], woT_d.rearrange("(eo p) d -> p eo d", p=128))
        p1_ctx.close()   # xT/wqkvT/fr/fi + projection PSUM no longer needed

        # ---------- phase 2 pools (reuse the PSUM banks phase 1 freed) ----
        psum = ctx.enter_context(tc.tile_pool(name="psum", bufs=2,
                                              space="PSUM"))
        pacc = ctx.enter_context(tc.tile_pool(name="pacc", bufs=1,
                                              space="PSUM"))
        pout = ctx.enter_context(tc.tile_pool(name="pout", bufs=2,
                                              space="PSUM"))

        # ---------- out-projection m-tile unit (PE filler work) ----------
        # evacuation alternates DVE/ACT so consecutive m-tiles' PSUM-bank
        # releases overlap (the dense tail is otherwise evacuation-paced)
        evac_flip = [0]

        def outproj_mtile(qt, m):
            qs = slice(qt * TT, (qt + 1) * TT)
            ops = pout.tile([128, TT], f32, tag="oproj", name="ops")
            for e in range(HEADS_PER_CORE):
                nc.tensor.matmul(ops[:],
                                 woT[:, e, m * 128:(m + 1) * 128],
                                 yT[e][:, qs], start=(e == 0),
                                 stop=(e == HEADS_PER_CORE - 1))
            osb = osbp.tile([128, TT], bf16, tag="osb", name="osb")
            if evac_flip[0] % 2:
                nc.scalar.copy(osb[:], ops[:])
            else:
                nc.vector.tensor_copy(osb[:], ops[:])
            evac_flip[0] += 1
            nc.sync.dma_start(outT_d[m * 128:(m + 1) * 128, qs], osb[:])

        pending = []     # out-proj m-tiles ready to weave into the PE stream

        def weave_one():
            if pending:
                outproj_mtile(*pending.pop(0))

        # ---------- attention unit (head h, query tile qt) ----------
        # Two key-tiles of scores share one 2-bank fp32 PSUM tile and one
        # exp instruction (halves the per-exp overhead), and the denom/y
        # consumer matmuls are software-pipelined LAG pairs behind the
        # score matmuls: the exp latency is then never on the PE critical
        # path, and yps/dps single-buffer without boundary stalls (a score
        # PSUM tile frees at its exp, not at its consumers).
        LAG = 2

        def attention_unit(h, qt, weave=0):
            ntk = 4 * (qt + 1)
            npair = ntk // 2
            dps = prow.tile([1, TT], f32, tag="row", name="dps")
            yps = pacc.tile([128, TT], f32, tag="yacc", name="yps")
            stages = []    # per-pair consumer args: (e, halves)

            def emit_scores(p):
                sps = psum.tile([128, 2, TT], f32, tag="mm", name="sps")
                halves = []
                for i in (0, 1):
                    # diagonal tiles (r >= 1) only have valid scores in
                    # their last TT - 128*r columns; skip the fully-masked
                    # prefix.  In suffix-local coords the causal mask is
                    # always the r=0 triangle.
                    tk = 2 * p + i
                    r = tk - 4 * qt
                    off = 128 * r if r > 0 else 0
                    qs = slice(qt * TT + off, (qt + 1) * TT)
                    nc.tensor.matmul(sps[:, i, off:],
                                     krot[:, tk * 128:(tk + 1) * 128],
                                     qrot[h][:, qs], start=True, stop=True)
                    halves.append((tk, r, off, TT - off))
                e = epool.tile([128, 2, TT], bf16, tag="e", name="e")
                if halves[0][1] < 0 and halves[1][1] < 0:
                    nc.scalar.activation(e[:], sps[:],
                                         mybir.ActivationFunctionType.Exp,
                                         bias=0.0, scale=SCALE)
                else:
                    for i, (tk, r, off, w) in enumerate(halves):
                        nc.scalar.activation(
                            e[:, i, off:], sps[:, i, off:],
                            mybir.ActivationFunctionType.Exp,
                            bias=0.0, scale=SCALE)
                stages.append((e, halves))

            def emit_consumers(p):
                e, halves = stages[p]
                full = halves[0][1] < 0 and halves[1][1] < 0
                if full:
                    # one denominator matmul per pair on the DVE pair-sum
                    # (single bf16 rounding of the pair, not a running sum)
                    e01 = epool.tile([128, TT], bf16, tag="em", name="e01")
                    nc.vector.tensor_add(e01[:], e[:, 0, :], e[:, 1, :])
                    nc.tensor.matmul(dps[:], ones[:], e01[:],
                                     start=(halves[0][0] == 0),
                                     stop=(halves[1][0] == ntk - 1))
                for i, (tk, r, off, w) in enumerate(halves):
                    src = e[:, i, off:]
                    if r >= 0:
                        em = epool.tile([128, TT], bf16, tag="em", name="em")
                        nc.vector.tensor_mul(em[:, :w], e[:, i, off:],
                                             cmask[:, :w])
                        src = em[:, :w]
                        nc.tensor.matmul(dps[:, off:], ones[:], src,
                                         start=(tk == 0),
                                         stop=(tk == ntk - 1))
                    elif not full:
                        nc.tensor.matmul(dps[:, off:], ones[:], src,
                                         start=(tk == 0),
                                         stop=(tk == ntk - 1))
                    nc.tensor.matmul(yps[:, off:], vtok[:, tk, :], src,
                                     start=(tk == 0), stop=(tk == ntk - 1))

            for p in range(npair):
                emit_scores(p)
                if p >= LAG:
                    emit_consumers(p - LAG)
                # drop an always-ready out-proj unit into the PE queue so
                # the engine has work while the next exp drains
                for _ in range(weave):
                    weave_one()
            for p in range(max(0, npair - LAG), npair):
                emit_consumers(p)
                # filler between the flushed consumers: their exps are the
                # freshest and otherwise stall the PE at unit boundaries
                if weave:
                    weave_one()
            qs = slice(qt * TT, (qt + 1) * TT)
            dr = sb.tile([1, TT], f32, tag="dr", name="dr")
            nc.vector.reciprocal_approx_fast(dr[:], dps[:])
            drb = sb.tile([128, TT], f32, tag="drb", name="drb")
            nc.gpsimd.partition_broadcast(drb[:], dr[:])
            nc.vector.tensor_mul(yT[h][:, qs], yps[:], drb[:])

        # ---------- emission: attention qt=3..0 with woven out-proj -------
        # out-proj for query tile qt becomes available once all 4 heads of
        # qt are done; it is woven into the following qt's attention.
        for qt in (3, 2, 1, 0):
            # more weave slots as attention units shrink
            weave = {3: 0, 2: 1, 1: 1, 0: 2}[qt]
            for h in range(HEADS_PER_CORE):
                attention_unit(h, qt, weave=weave)
            pending.extend((qt, m) for m in range(D // 128))
        while pending:
            weave_one()

    nc.compile()
    return nc


def _host_shards(x, freqs_cis, wqkv, wo):
    import ml_dtypes
    bf16 = ml_dtypes.bfloat16

    # head-dim permutation: even dims then odd dims (for q and k only)
    perm = np.concatenate([np.arange(0, HD, 2), np.arange(1, HD, 2)])

    wq = wqkv[:Q_SIZE].reshape(NH, HD, D)[:, perm, :]
    wk = wqkv[Q_SIZE:Q_SIZE + KV_SIZE].reshape(NKV, HD, D)[:, perm, :]
    wv = wqkv[Q_SIZE + KV_SIZE:].reshape(NKV, HD, D)

    fr1 = np.ascontiguousarray(freqs_cis[:, :, 0].T, dtype=np.float32)
    fi1 = np.ascontiguousarray(freqs_cis[:, :, 1].T, dtype=np.float32)
    fr = np.vstack([fr1, fr1])
    # sign baked in so the rotary combine is a single add:
    #   rot[lo] = q_lo*fr + q_hi*(-fi) ; rot[hi] = q_hi*fr + q_lo*(+fi)
    fi = np.vstack([fi1, -fi1])

    # causal mask for the leading diagonal of a 128-row x 512-col score
    # tile (suffix-narrowed diagonal tiles reuse its prefix columns)
    tkl = np.arange(128)[:, None]
    tql = np.arange(TT)[None, :]
    mask = (tkl <= tql).astype(bf16)

    in_maps = []
    for c in range(N_CORES):
        b, j = divmod(c, TPC)
        wshard = np.concatenate(
            [wq[TPC * j + h] for h in range(HEADS_PER_CORE)] +
            [wk[j], wv[j]], axis=0)                     # (768, D)
        # [chunk, p, ko, e] with d = ko*128 + p
        wpack = np.ascontiguousarray(
            wshard.reshape(HEADS_PER_CORE + 2, HD, N_KT, 128)
            .transpose(0, 3, 2, 1)).astype(bf16)
        in_maps.append({
            "xT": np.ascontiguousarray(x[b].T).astype(bf16),
            "wqkvT": wpack,
            "woT": np.ascontiguousarray(
                wo[:, j * E_LOC:(j + 1) * E_LOC].T).astype(bf16),
            "fr": fr,
            "fi": fi,
            "mask": mask,
        })
    return in_maps


_NC_CACHE = {}


def _get_nc():
    if "nc" not in _NC_CACHE:
        _NC_CACHE["nc"] = _build_bass()
    return _NC_CACHE["nc"]


def kernel(x, freqs_cis, wqkv, wo, q_norm_w, k_norm_w, _want_results=False):
    # q_norm_w / k_norm_w are all-ones per the problem spec; rmsnorm weight
    # multiply is the identity and is folded away.
    from concourse.bass_utils import run_bass_kernel_spmd

    nc = _get_nc()
    in_maps = _host_shards(np.asarray(x, np.float32),
                           np.asarray(freqs_cis, np.float32),
                           np.asarray(wqkv, np.float32),
                           np.asarray(wo, np.float32))
    res = run_bass_kernel_spmd(nc, in_maps, core_ids=list(range(N_CORES)))
    parts = [r["outT"] for r in res.results]
    out = np.empty((B, S, D), np.float32)
    for b in range(B):
        acc = parts[TPC * b].astype(np.float32)
        for j in range(1, TPC):
            acc = acc + parts[TPC * b + j].astype(np.float32)
        out[b] = acc.T
    if _want_results:
        return out, res
    return out


# revision 26
# speedup vs baseline: 1.1922x; 1.0275x over previous
"""Trainium2 Bass kernel for nn_Attention (dense transformer attention block).

Reference semantics (B=2, S=2048, D=2048, NH=16, NKV=4, HD=128):
    qkv = x @ wqkv.T ; split q/k/v ; rmsnorm(q), rmsnorm(k) (weights == 1)
    rotary(q), rotary(k) with arbitrary freqs_cis ; GQA repeat kv 4x
    causal softmax attention ; out = y @ wo.T

Sharding: 2-way data parallel over batch x 4-way tensor parallel over head
groups (each core owns 4 query heads + their single shared KV head).  Each
core computes a full-shape partial of the output projection for its batch
element; the host sums the 4 partials per batch element (the "all-reduce").

Device layout notes:
  - All tensors are kept "transposed" (feature dim on partitions, tokens on
    the free dim) so every matmul chains without transposes; only V is
    transposed on-device (PE transpose) to the [token, dv] layout the
    attention-value matmul needs as its stationary operand.
  - Head dims are permuted (even dims then odd dims) on the host so rotary
    becomes two contiguous 64-partition slabs; the permutation cancels in
    the q.k contraction and V/wo are left in natural order.
  - Softmax uses no running max: |scores| <= sqrt(128)*max|f|^2 is far below
    exp overflow in fp32 (verified empirically), so exp() is applied
    directly and the denominator is accumulated with a ones-vector matmul.
  - Rotary intermediates are bf16 so the vector engine runs in its 2x
    packed mode; output partials are stored bf16 (host accumulates fp32).
  - The out-projection is emitted as per-m-tile units woven into the NEXT
    query tile's attention stream: they are always-ready PE work placed at
    the points where the attention chain waits on exp, so the FIFO engine
    queue fills what would otherwise be stalls.
"""

import math
from contextlib import ExitStack

import numpy as np

B, S, D = 2, 2048, 2048
NH, NKV, HD = 16, 4, 128
EPS = 1e-6
N_CORES = 8
TPC = 4            # tensor-parallel cores per batch element
HEADS_PER_CORE = NH // TPC          # 4
Q_SIZE, KV_SIZE = NH * HD, NKV * HD
E_LOC = HEADS_PER_CORE * HD         # 512 local y/e dims per core
TT = 512                            # token tile (free dim) for matmuls
N_TT = S // TT                      # 4
N_KT = D // 128                     # 16 contraction tiles for projections
N_SKT = S // 128                    # 16 key tiles per sequence

_F32 = "float32"


def _steer_act_tables():
    """Make Exp and Ln both resolve to the combined natural_log_exp table.

    bacc's insert_act_table_loads picks the first act-function set that
    contains each function, which puts Exp and Ln in different tables and
    costs a ~1.3us ACT table re-load on every rmsnorm <-> softmax switch.
    Stripping Exp/Ln from the other sets (list positions preserved, so set
    ids stay valid for walrus) leaves one shared table and a single load.
    """
    from concourse import bacc
    import concourse.mybir as mybir
    import concourse.hw_specs as hw_specs

    if getattr(bacc.get_activation_tables, "_act_steered", False):
        return
    orig = hw_specs.get_activation_tables

    def steered(arch):
        tabs = orig(arch)
        for name, fns in tabs.items():
            if name != "natural_log_exp_and_others":
                fns.discard(mybir.ActivationFunctionType.Exp)
                fns.discard(mybir.ActivationFunctionType.Ln)
        return tabs

    steered._act_steered = True
    bacc.get_activation_tables = steered


def _build_bass():
    import concourse.bass as bass  # noqa: F401
    import concourse.mybir as mybir
    import concourse.tile as tile
    from concourse import bacc
    from concourse.masks import make_identity

    _steer_act_tables()

    f32 = mybir.dt.float32
    f16 = mybir.dt.float16
    bf16 = mybir.dt.bfloat16

    nc = bacc.Bacc("TRN2", target_bir_lowering=False, debug=False,
                   num_devices=N_CORES)

    # ---- DRAM I/O (per-core shards supplied via in_maps) ----
    xT_d = nc.dram_tensor("xT", (D, S), bf16, kind="ExternalInput").ap()
    # per-chunk-contiguous weight layout: [chunk, p, ko, e] so each chunk's
    # stationary tiles stream in with 4 KiB/partition contiguous lines
    wqkvT_d = nc.dram_tensor(
        "wqkvT", (HEADS_PER_CORE + 2, 128, N_KT, HD), bf16,
        kind="ExternalInput").ap()
    woT_d = nc.dram_tensor("woT", (E_LOC, D), bf16, kind="ExternalInput").ap()
    # fr/fi are duplicated across both 64-partition halves so rotary ops can
    # pair them with either the even (base 0) or odd (base 64) slab of q/k
    fr_d = nc.dram_tensor("fr", (HD, S), f32, kind="ExternalInput").ap()
    fi_d = nc.dram_tensor("fi", (HD, S), f32, kind="ExternalInput").ap()
    mask_d = nc.dram_tensor("mask", (128, TT), bf16,
                            kind="ExternalInput").ap()
    outT_d = nc.dram_tensor("outT", (D, S), bf16, kind="ExternalOutput").ap()

    NCHUNK = HEADS_PER_CORE + 2     # 4 q heads, 1 k head, 1 v head
    SCALE = 1.0 / math.sqrt(HD)

    with tile.TileContext(nc) as tc, ExitStack() as ctx:
        # ---------- pools ----------
        const = ctx.enter_context(tc.tile_pool(name="const", bufs=1))
        sb = ctx.enter_context(tc.tile_pool(name="sb", bufs=2))
        # output staging is deep: each slot is held through its DMA's ~2us
        # HBM completion receipt, and the out-proj tail is evacuation-paced
        osbp = ctx.enter_context(tc.tile_pool(name="osbp", bufs=6))
        epool = ctx.enter_context(tc.tile_pool(name="epool", bufs=4))
        prow = ctx.enter_context(tc.tile_pool(name="prow", bufs=1,
                                              space="PSUM"))

        # ---------- resident tensors ----------
        # phase-1-only tensors live in their own pool, freed before attention
        # needs peak SBUF
        p1_ctx = ExitStack()
        p1 = p1_ctx.enter_context(tc.tile_pool(name="p1", bufs=1))
        p1w = p1_ctx.enter_context(tc.tile_pool(name="p1w", bufs=3))
        pproj = p1_ctx.enter_context(tc.tile_pool(name="pproj", bufs=5,
                                                  space="PSUM"))
        ptp = p1_ctx.enter_context(tc.tile_pool(name="ptp", bufs=2,
                                                space="PSUM"))
        # DMA priority: the k-chunk weights + first xT tiles gate the first
        # matmul, so they go first; bulk/constant loads go on the gpsimd
        # DMA queue so their issue cost doesn't delay the critical loads.
        xT = p1.tile([128, N_KT, S], bf16)               # 64 KiB/part
        xT_r = xT_d.rearrange("(ko p) t -> p ko t", p=128)
        fr = p1.tile([HD, S], f32)
        fi = p1.tile([HD, S], f32)
        woT = const.tile([128, HEADS_PER_CORE, D], bf16)
        cmask = const.tile([128, TT], bf16)

        def load_wch(chunk):
            wch = p1w.tile([128, N_KT, HD], bf16, tag="wch", name="wch")
            nc.sync.dma_start(wch[:], wqkvT_d[chunk])
            return wch

        wch_next = load_wch(HEADS_PER_CORE)    # k-chunk weights first
        wch_v = None
        for half in range(2):
            hs = slice(half * (S // 2), (half + 1) * (S // 2))
            for kt in range(N_KT):
                # stream x in the order the projection consumes it: all k
                # tiles of the first token half, then the second half
                nc.sync.dma_start(xT[:, kt, hs], xT_r[:, kt, hs])
                if half == 0 and kt == 1:
                    # v weights slot between the first x pieces so the v
                    # tiles can join the ramp group almost immediately
                    wch_v = load_wch(HEADS_PER_CORE + 1)
        nc.gpsimd.dma_start(fr[:], fr_d)
        nc.gpsimd.dma_start(fi[:], fi_d)
        nc.gpsimd.dma_start(cmask[:], mask_d)

        ident = const.tile([128, 128], bf16)
        make_identity(nc, ident[:])
        ones = const.tile([128, 1], bf16)
        nc.vector.memset(ones[:], 1.0)
        epsb = const.tile([1, 1], f32)
        nc.vector.memset(epsb[:], EPS)

        # rotated q (4 heads), rotated k, and v in [token, dv] layout
        qrot = [const.tile([128, S], bf16, tag=f"qrot{h}", name=f"qrot{h}")
                for h in range(HEADS_PER_CORE)]
        krot = const.tile([128, S], bf16)
        vT = const.tile([128, S], bf16)
        vtok = const.tile([128, N_SKT, HD], bf16)
        # normalized attention outputs (yT), stationary input of out-proj
        yT = [const.tile([128, S], bf16, tag=f"yT{h}", name=f"yT{h}")
              for h in range(HEADS_PER_CORE)]

        # ---------- phase 1 helpers: projection chunks (+norm+rotary) ------
        def proj_consume(chunk, tt, ps):
            is_v = chunk == HEADS_PER_CORE + 1
            is_k = chunk == HEADS_PER_CORE
            ts = slice(tt * TT, (tt + 1) * TT)
            if is_v:
                nc.vector.tensor_copy(vT[:, ts], ps[:])
            else:
                # rms stats: mean over head dim (partitions) via ones-matmul;
                # square runs on ACT (same table set as exp/ln)
                sq = sb.tile([128, TT], bf16, tag="sq", name="sq")
                nc.scalar.activation(sq[:], ps[:],
                                     mybir.ActivationFunctionType.Square)
                ms = prow.tile([1, TT], f32, tag="row", name="ms")
                nc.tensor.matmul(ms[:], ones[:], sq[:], start=True, stop=True)
                lnms = sb.tile([1, TT], f32, tag="lnms", name="lnms")
                nc.scalar.activation(lnms[:], ms[:],
                                     mybir.ActivationFunctionType.Ln,
                                     bias=epsb[:], scale=1.0 / HD)
                rs = sb.tile([1, TT], f32, tag="rs", name="rs")
                nc.scalar.activation(rs[:], lnms[:],
                                     mybir.ActivationFunctionType.Exp,
                                     bias=0.0, scale=-0.5)
                rsb = sb.tile([128, TT], f32, tag="rsb", name="rsb")
                nc.gpsimd.partition_broadcast(rsb[:], rs[:])
                # rotary, even dims on partitions 0:64, odd on 64:128:
                #   a      = q * fr            (both halves at once)
                #   bswap  = swap_halves(q) * [+fi; -fi]  (2 cross-half muls;
                #            the sign baked into fi makes the combine an add)
                #   rot    = a + bswap
                # kept fp32 until the final bf16 store: the rounding error
                # of bf16 intermediates is amplified ~|score| by the exp
                rot = sb.tile([128, TT], f32, tag="rot", name="rot")
                a = sb.tile([128, TT], f32, tag="rota", name="a")
                nc.vector.tensor_mul(a[:], ps[:], fr[:, ts])
                bsw = sb.tile([128, TT], f32, tag="rotb", name="bsw")
                nc.vector.tensor_mul(bsw[0:64, :], ps[64:128, :],
                                     fi[64:128, ts])
                nc.vector.tensor_mul(bsw[64:128, :], ps[0:64, :],
                                     fi[0:64, ts])
                nc.vector.tensor_add(rot[:], a[:], bsw[:])
                dst = krot if is_k else qrot[chunk]
                nc.vector.tensor_mul(dst[:, ts], rot[:], rsb[:])

        def project_chunk(chunk, wch=None):
            if wch is None:
                wch = load_wch(chunk)
            for tt in range(N_TT):
                ts = slice(tt * TT, (tt + 1) * TT)
                ps = pproj.tile([128, TT], f32, tag="proj", name="ps")
                for kt in range(N_KT):
                    nc.tensor.matmul(
                        ps[:], wch[:, kt, :],
                        xT[:, kt, ts], start=(kt == 0), stop=(kt == N_KT - 1))
                proj_consume(chunk, tt, ps)

        # ---------- emission: projections ----------
        # The k chunk runs while x is still streaming in: tiles are
        # interleaved with a phase lag so the lagging tiles' matmuls
        # (whose x pieces arrived long ago) fill the DMA arrival gaps
        # that otherwise idle the PE during the ramp, and the tiles
        # finish staggered so their rms/rotary consumer chains pipeline.
        # The half-1 ramp group adds the two half-0 v tiles (x resident,
        # weights preloaded) as always-ready filler.
        LAGK = 3
        KCH, VCH = HEADS_PER_CORE, HEADS_PER_CORE + 1

        def ramp_group(plan):
            # plan: list of (chunk, wch, tt, lag)
            pss, tss = [], []
            for chunk, wch, tt, lag in plan:
                pss.append(pproj.tile([128, TT], f32, tag="proj", name="ps"))
                tss.append(slice(tt * TT, (tt + 1) * TT))
            maxlag = max(lag for _, _, _, lag in plan)
            for r in range(N_KT + maxlag):
                for (chunk, wch, tt, lag), ps, ts in zip(plan, pss, tss):
                    kt = r - lag
                    if 0 <= kt < N_KT:
                        nc.tensor.matmul(
                            ps[:], wch[:, kt, :], xT[:, kt, ts],
                            start=(kt == 0), stop=(kt == N_KT - 1))
            for (chunk, wch, tt, lag), ps in zip(plan, pss):
                proj_consume(chunk, tt, ps)

        ramp_group([(KCH, wch_next, 0, 0), (KCH, wch_next, 1, 2),
                    (VCH, wch_v, 0, 3), (VCH, wch_v, 1, 4)])
        ramp_group([(KCH, wch_next, 2, 0), (KCH, wch_next, 3, 2),
                    (VCH, wch_v, 2, 3), (VCH, wch_v, 3, 4)])
        for h in range(HEADS_PER_CORE):
            project_chunk(h)
        # v -> [token, dv] layout; emitted last so these ready-to-run PE ops
        # fill the gap while the final q chunk's rotary drains the PSUM
        # banks the attention pools alias
        for i in range(N_SKT):
            tp = ptp.tile([128, 128], bf16, tag="tp", name="tp")
            nc.tensor.transpose(tp[:], vT[:, i * 128:(i + 1) * 128], ident[:])
            nc.vector.tensor_copy(vtok[:, i, :], tp[:])
        # wo stream goes last on the sync queue: it is only needed by the
        # first out-projection (~40us later) and must not steal HBM
        # bandwidth from the x/wqkv stream that gates phase 1.
        nc.sync.dma_start(
            woT[:], woT_d.rearrange("(eo p) d -> p eo d", p=128))
        p1_ctx.close()   # xT/wqkvT/fr/fi + projection PSUM no longer needed

        # ---------- phase 2 pools (reuse the PSUM banks phase 1 freed) ----
        psum = ctx.enter_context(tc.tile_pool(name="psum", bufs=2,
                                              space="PSUM"))
        pacc = ctx.enter_context(tc.tile_pool(name="pacc", bufs=1,
                                              space="PSUM"))
        pout = ctx.enter_context(tc.tile_pool(name="pout", bufs=2,
                                              space="PSUM"))

        # ---------- out-projection m-tile unit (PE filler work) ----------
        # evacuation alternates DVE/ACT so consecutive m-tiles' PSUM-bank
        # releases overlap (the dense tail is otherwise evacuation-paced)
        evac_flip = [0]

        def outproj_mtile(qt, m):
            qs = slice(qt * TT, (qt + 1) * TT)
            ops = pout.tile([128, TT], f32, tag="oproj", name="ops")
            for e in range(HEADS_PER_CORE):
                nc.tensor.matmul(ops[:],
                                 woT[:, e, m * 128:(m + 1) * 128],
                                 yT[e][:, qs], start=(e == 0),
                                 stop=(e == HEADS_PER_CORE - 1))
            osb = osbp.tile([128, TT], bf16, tag="osb", name="osb")
            if evac_flip[0] % 2:
                nc.scalar.copy(osb[:], ops[:])
            else:
                nc.vector.tensor_copy(osb[:], ops[:])
            evac_flip[0] += 1
            nc.sync.dma_start(outT_d[m * 128:(m + 1) * 128, qs], osb[:])

        pending = []     # out-proj m-tiles ready to weave into the PE stream

        def weave_one():
            if pending:
                outproj_mtile(*pending.pop(0))

        # ---------- attention unit (head h, query tile qt) ----------
        # Two key-tiles of scores share one 2-bank fp32 PSUM tile and one
        # exp instruction (halves the per-exp overhead), and the denom/y
        # consumer matmuls are software-pipelined LAG pairs behind the
        # score matmuls: the exp latency is then never on the PE critical
        # path, and yps/dps single-buffer without boundary stalls (a score
        # PSUM tile frees at its exp, not at its consumers).
        LAG = 2

        def attention_unit(h, qt, weave=0):
            ntk = 4 * (qt + 1)
            npair = ntk // 2
            dps = prow.tile([1, TT], f32, tag="row", name="dps")
            yps = pacc.tile([128, TT], f32, tag="yacc", name="yps")
            stages = []    # per-pair consumer args: (e, halves)

            def emit_scores(p):
                sps = psum.tile([128, 2, TT], f32, tag="mm", name="sps")
                halves = []
                for i in (0, 1):
                    # diagonal tiles (r >= 1) only have valid scores in
                    # their last TT - 128*r columns; skip the fully-masked
                    # prefix.  In suffix-local coords the causal mask is
                    # always the r=0 triangle.
                    tk = 2 * p + i
                    r = tk - 4 * qt
                    off = 128 * r if r > 0 else 0
                    qs = slice(qt * TT + off, (qt + 1) * TT)
                    nc.tensor.matmul(sps[:, i, off:],
                                     krot[:, tk * 128:(tk + 1) * 128],
                                     qrot[h][:, qs], start=True, stop=True)
                    halves.append((tk, r, off, TT - off))
                e = epool.tile([128, 2, TT], bf16, tag="e", name="e")
                if halves[0][1] < 0 and halves[1][1] < 0:
                    nc.scalar.activation(e[:], sps[:],
                                         mybir.ActivationFunctionType.Exp,
                                         bias=0.0, scale=SCALE)
                else:
                    for i, (tk, r, off, w) in enumerate(halves):
                        nc.scalar.activation(
                            e[:, i, off:], sps[:, i, off:],
                            mybir.ActivationFunctionType.Exp,
                            bias=0.0, scale=SCALE)
                stages.append((e, halves))

            def emit_consumers(p):
                e, halves = stages[p]
                full = halves[0][1] < 0 and halves[1][1] < 0
                if full:
                    # one denominator matmul per pair on the DVE pair-sum
                    # (single bf16 rounding of the pair, not a running sum)
                    e01 = epool.tile([128, TT], bf16, tag="em", name="e01")
                    nc.vector.tensor_add(e01[:], e[:, 0, :], e[:, 1, :])
                    nc.tensor.matmul(dps[:], ones[:], e01[:],
                                     start=(halves[0][0] == 0),
                                     stop=(halves[1][0] == ntk - 1))
                for i, (tk, r, off, w) in enumerate(halves):
                    src = e[:, i, off:]
                    if r >= 0:
                        em = epool.tile([128, TT], bf16, tag="em", name="em")
                        nc.vector.tensor_mul(em[:, :w], e[:, i, off:],
                                             cmask[:, :w])
                        src = em[:, :w]
                        nc.tensor.matmul(dps[:, off:], ones[:], src,
                                         start=(tk == 0),
                                         stop=(tk == ntk - 1))
                    elif not full:
                        nc.tensor.matmul(dps[:, off:], ones[:], src,
                                         start=(tk == 0),
                                         stop=(tk == ntk - 1))
                    nc.tensor.matmul(yps[:, off:], vtok[:, tk, :], src,
                                     start=(tk == 0), stop=(tk == ntk - 1))

            for p in range(npair):
                emit_scores(p)
                if p >= LAG:
                    emit_consumers(p - LAG)
                # drop an always-ready out-proj unit into the PE queue so
                # the engine has work while the next exp drains
                for _ in range(weave):
                    weave_one()
            for p in range(max(0, npair - LAG), npair):
                emit_consumers(p)
                # filler between the flushed consumers: their exps are the
                # freshest and otherwise stall the PE at unit boundaries
                if weave:
                    weave_one()
            qs = slice(qt * TT, (qt + 1) * TT)
            dr = sb.tile([1, TT], f32, tag="dr", name="dr")
            nc.vector.reciprocal_approx_fast(dr[:], dps[:])
            drb = sb.tile([128, TT], f32, tag="drb", name="drb")
            nc.gpsimd.partition_broadcast(drb[:], dr[:])
            nc.vector.tensor_mul(yT[h][:, qs], yps[:], drb[:])

        # ---------- emission: attention qt=3..0 with woven out-proj -------
        # out-proj for query tile qt becomes available once all 4 heads of
        # qt are done; it is woven into the following qt's attention.
        for qt in (3, 2, 1, 0):
            # more weave slots as attention units shrink
            weave = {3: 0, 2: 1, 1: 1, 0: 1}[qt]
            for h in range(HEADS_PER_CORE):
                attention_unit(h, qt, weave=weave)
            pending.extend((qt, m) for m in range(D // 128))
        while pending:
            weave_one()

    nc.compile()
    return nc


def _host_shards(x, freqs_cis, wqkv, wo):
    import ml_dtypes
    bf16 = ml_dtypes.bfloat16

    # head-dim permutation: even dims then odd dims (for q and k only)
    perm = np.concatenate([np.arange(0, HD, 2), np.arange(1, HD, 2)])

    wq = wqkv[:Q_SIZE].reshape(NH, HD, D)[:, perm, :]
    wk = wqkv[Q_SIZE:Q_SIZE + KV_SIZE].reshape(NKV, HD, D)[:, perm, :]
    wv = wqkv[Q_SIZE + KV_SIZE:].reshape(NKV, HD, D)

    fr1 = np.ascontiguousarray(freqs_cis[:, :, 0].T, dtype=np.float32)
    fi1 = np.ascontiguousarray(freqs_cis[:, :, 1].T, dtype=np.float32)
    fr = np.vstack([fr1, fr1])
    # sign baked in so the rotary combine is a single add:
    #   rot[lo] = q_lo*fr + q_hi*(-fi) ; rot[hi] = q_hi*fr + q_lo*(+fi)
    fi = np.vstack([fi1, -fi1])

    # causal mask for the leading diagonal of a 128-row x 512-col score
    # tile (suffix-narrowed diagonal tiles reuse its prefix columns)
    tkl = np.arange(128)[:, None]
    tql = np.arange(TT)[None, :]
    mask = (tkl <= tql).astype(bf16)

    in_maps = []
    for c in range(N_CORES):
        b, j = divmod(c, TPC)
        wshard = np.concatenate(
            [wq[TPC * j + h] for h in range(HEADS_PER_CORE)] +
            [wk[j], wv[j]], axis=0)                     # (768, D)
        # [chunk, p, ko, e] with d = ko*128 + p
        wpack = np.ascontiguousarray(
            wshard.reshape(HEADS_PER_CORE + 2, HD, N_KT, 128)
            .transpose(0, 3, 2, 1)).astype(bf16)
        in_maps.append({
            "xT": np.ascontiguousarray(x[b].T).astype(bf16),
            "wqkvT": wpack,
            "woT": np.ascontiguousarray(
                wo[:, j * E_LOC:(j + 1) * E_LOC].T).astype(bf16),
            "fr": fr,
            "fi": fi,
            "mask": mask,
        })
    return in_maps


_NC_CACHE = {}


def _get_nc():
    if "nc" not in _NC_CACHE:
        _NC_CACHE["nc"] = _build_bass()
    return _NC_CACHE["nc"]


def kernel(x, freqs_cis, wqkv, wo, q_norm_w, k_norm_w, _want_results=False):
    # q_norm_w / k_norm_w are all-ones per the problem spec; rmsnorm weight
    # multiply is the identity and is folded away.
    from concourse.bass_utils import run_bass_kernel_spmd

    nc = _get_nc()
    in_maps = _host_shards(np.asarray(x, np.float32),
                           np.asarray(freqs_cis, np.float32),
                           np.asarray(wqkv, np.float32),
                           np.asarray(wo, np.float32))
    res = run_bass_kernel_spmd(nc, in_maps, core_ids=list(range(N_CORES)))
    parts = [r["outT"] for r in res.results]
    out = np.empty((B, S, D), np.float32)
    for b in range(B):
        acc = parts[TPC * b].astype(np.float32)
        for j in range(1, TPC):
            acc = acc + parts[TPC * b + j].astype(np.float32)
        out[b] = acc.T
    if _want_results:
        return out, res
    return out


# revision 29
# speedup vs baseline: 1.2085x; 1.0137x over previous
"""Trainium2 Bass kernel for nn_Attention (dense transformer attention block).

Reference semantics (B=2, S=2048, D=2048, NH=16, NKV=4, HD=128):
    qkv = x @ wqkv.T ; split q/k/v ; rmsnorm(q), rmsnorm(k) (weights == 1)
    rotary(q), rotary(k) with arbitrary freqs_cis ; GQA repeat kv 4x
    causal softmax attention ; out = y @ wo.T

Sharding: 2-way data parallel over batch x 4-way tensor parallel over head
groups (each core owns 4 query heads + their single shared KV head).  Each
core computes a full-shape partial of the output projection for its batch
element; the host sums the 4 partials per batch element (the "all-reduce").

Device layout notes:
  - All tensors are kept "transposed" (feature dim on partitions, tokens on
    the free dim) so every matmul chains without transposes; only V is
    transposed on-device (PE transpose) to the [token, dv] layout the
    attention-value matmul needs as its stationary operand.
  - Head dims are permuted (even dims then odd dims) on the host so rotary
    becomes two contiguous 64-partition slabs; the permutation cancels in
    the q.k contraction and V/wo are left in natural order.
  - Softmax uses no running max: |scores| <= sqrt(128)*max|f|^2 is far below
    exp overflow in fp32 (verified empirically), so exp() is applied
    directly and the denominator is accumulated with a ones-vector matmul.
  - Rotary intermediates are bf16 so the vector engine runs in its 2x
    packed mode; output partials are stored bf16 (host accumulates fp32).
  - The out-projection is emitted as per-m-tile units woven into the NEXT
    query tile's attention stream: they are always-ready PE work placed at
    the points where the attention chain waits on exp, so the FIFO engine
    queue fills what would otherwise be stalls.
"""

import math
from contextlib import ExitStack

import numpy as np

B, S, D = 2, 2048, 2048
NH, NKV, HD = 16, 4, 128
EPS = 1e-6
N_CORES = 8
TPC = 4            # tensor-parallel cores per batch element
HEADS_PER_CORE = NH // TPC          # 4
Q_SIZE, KV_SIZE = NH * HD, NKV * HD
E_LOC = HEADS_PER_CORE * HD         # 512 local y/e dims per core
TT = 512                            # token tile (free dim) for matmuls
N_TT = S // TT                      # 4
N_KT = D // 128                     # 16 contraction tiles for projections
N_SKT = S // 128                    # 16 key tiles per sequence

_F32 = "float32"


def _steer_act_tables():
    """Make Exp and Ln both resolve to the combined natural_log_exp table.

    bacc's insert_act_table_loads picks the first act-function set that
    contains each function, which puts Exp and Ln in different tables and
    costs a ~1.3us ACT table re-load on every rmsnorm <-> softmax switch.
    Stripping Exp/Ln from the other sets (list positions preserved, so set
    ids stay valid for walrus) leaves one shared table and a single load.
    """
    from concourse import bacc
    import concourse.mybir as mybir
    import concourse.hw_specs as hw_specs

    if getattr(bacc.get_activation_tables, "_act_steered", False):
        return
    orig = hw_specs.get_activation_tables

    def steered(arch):
        tabs = orig(arch)
        for name, fns in tabs.items():
            if name != "natural_log_exp_and_others":
                fns.discard(mybir.ActivationFunctionType.Exp)
                fns.discard(mybir.ActivationFunctionType.Ln)
        return tabs

    steered._act_steered = True
    bacc.get_activation_tables = steered


def _build_bass():
    import concourse.bass as bass  # noqa: F401
    import concourse.mybir as mybir
    import concourse.tile as tile
    from concourse import bacc
    from concourse.masks import make_identity

    _steer_act_tables()

    f32 = mybir.dt.float32
    f16 = mybir.dt.float16
    bf16 = mybir.dt.bfloat16

    nc = bacc.Bacc("TRN2", target_bir_lowering=False, debug=False,
                   num_devices=N_CORES)

    # ---- DRAM I/O (per-core shards supplied via in_maps) ----
    xT_d = nc.dram_tensor("xT", (D, S), bf16, kind="ExternalInput").ap()
    # per-chunk-contiguous weight layout: [chunk, p, ko, e] so each chunk's
    # stationary tiles stream in with 4 KiB/partition contiguous lines
    wqkvT_d = nc.dram_tensor(
        "wqkvT", (HEADS_PER_CORE + 2, 128, N_KT, HD), bf16,
        kind="ExternalInput").ap()
    woT_d = nc.dram_tensor("woT", (E_LOC, D), bf16, kind="ExternalInput").ap()
    # fr/fi are duplicated across both 64-partition halves so rotary ops can
    # pair them with either the even (base 0) or odd (base 64) slab of q/k
    fr_d = nc.dram_tensor("fr", (HD, S), f32, kind="ExternalInput").ap()
    fi_d = nc.dram_tensor("fi", (HD, S), f32, kind="ExternalInput").ap()
    mask_d = nc.dram_tensor("mask", (128, TT), bf16,
                            kind="ExternalInput").ap()
    outT_d = nc.dram_tensor("outT", (D, S), bf16, kind="ExternalOutput").ap()

    NCHUNK = HEADS_PER_CORE + 2     # 4 q heads, 1 k head, 1 v head
    SCALE = 1.0 / math.sqrt(HD)

    with tile.TileContext(nc) as tc, ExitStack() as ctx:
        # ---------- pools ----------
        const = ctx.enter_context(tc.tile_pool(name="const", bufs=1))
        sb = ctx.enter_context(tc.tile_pool(name="sb", bufs=2))
        # output staging is deep: each slot is held through its DMA's ~2us
        # HBM completion receipt, and the out-proj tail is evacuation-paced
        osbp = ctx.enter_context(tc.tile_pool(name="osbp", bufs=6))
        epool = ctx.enter_context(tc.tile_pool(name="epool", bufs=4))
        prow = ctx.enter_context(tc.tile_pool(name="prow", bufs=1,
                                              space="PSUM"))

        # ---------- resident tensors ----------
        # phase-1-only tensors live in their own pool, freed before attention
        # needs peak SBUF
        p1_ctx = ExitStack()
        p1 = p1_ctx.enter_context(tc.tile_pool(name="p1", bufs=1))
        p1w = p1_ctx.enter_context(tc.tile_pool(name="p1w", bufs=3))
        pproj = p1_ctx.enter_context(tc.tile_pool(name="pproj", bufs=5,
                                                  space="PSUM"))
        ptp = p1_ctx.enter_context(tc.tile_pool(name="ptp", bufs=2,
                                                space="PSUM"))
        # DMA priority: the k-chunk weights + first xT tiles gate the first
        # matmul, so they go first; bulk/constant loads go on the gpsimd
        # DMA queue so their issue cost doesn't delay the critical loads.
        xT = p1.tile([128, N_KT, S], bf16)               # 64 KiB/part
        xT_r = xT_d.rearrange("(ko p) t -> p ko t", p=128)
        fr = p1.tile([HD, S], f32)
        fi = p1.tile([HD, S], f32)
        woT = const.tile([128, HEADS_PER_CORE, D], bf16)
        cmask = const.tile([128, TT], bf16)

        def load_wch(chunk):
            wch = p1w.tile([128, N_KT, HD], bf16, tag="wch", name="wch")
            nc.sync.dma_start(wch[:], wqkvT_d[chunk])
            return wch

        wch_next = load_wch(HEADS_PER_CORE)    # k-chunk weights first
        wch_v = None
        for half in range(2):
            hs = slice(half * (S // 2), (half + 1) * (S // 2))
            for kt in range(N_KT):
                # stream x in the order the projection consumes it: all k
                # tiles of the first token half, then the second half
                nc.sync.dma_start(xT[:, kt, hs], xT_r[:, kt, hs])
                if half == 0 and kt == 1:
                    # v weights slot between the first x pieces so the v
                    # tiles can join the ramp group almost immediately
                    wch_v = load_wch(HEADS_PER_CORE + 1)
        nc.gpsimd.dma_start(fr[:], fr_d)
        nc.gpsimd.dma_start(fi[:], fi_d)
        nc.gpsimd.dma_start(cmask[:], mask_d)

        ident = const.tile([128, 128], bf16)
        make_identity(nc, ident[:])
        ones = const.tile([128, 1], bf16)
        nc.vector.memset(ones[:], 1.0)
        epsb = const.tile([1, 1], f32)
        nc.vector.memset(epsb[:], EPS)

        # rotated q (4 heads), rotated k, and v in [token, dv] layout
        qrot = [const.tile([128, S], bf16, tag=f"qrot{h}", name=f"qrot{h}")
                for h in range(HEADS_PER_CORE)]
        krot = const.tile([128, S], bf16)
        vT = const.tile([128, S], bf16)
        vtok = const.tile([128, N_SKT, HD], bf16)
        # normalized attention outputs (yT), stationary input of out-proj
        yT = [const.tile([128, S], bf16, tag=f"yT{h}", name=f"yT{h}")
              for h in range(HEADS_PER_CORE)]

        # ---------- phase 1 helpers: projection chunks (+norm+rotary) ------
        def proj_consume(chunk, tt, ps):
            is_v = chunk == HEADS_PER_CORE + 1
            is_k = chunk == HEADS_PER_CORE
            ts = slice(tt * TT, (tt + 1) * TT)
            if is_v:
                nc.vector.tensor_copy(vT[:, ts], ps[:])
            else:
                # rms stats: mean over head dim (partitions) via ones-matmul;
                # square runs on ACT (same table set as exp/ln)
                sq = sb.tile([128, TT], bf16, tag="sq", name="sq")
                nc.scalar.activation(sq[:], ps[:],
                                     mybir.ActivationFunctionType.Square)
                ms = prow.tile([1, TT], f32, tag="row", name="ms")
                nc.tensor.matmul(ms[:], ones[:], sq[:], start=True, stop=True)
                lnms = sb.tile([1, TT], f32, tag="lnms", name="lnms")
                nc.scalar.activation(lnms[:], ms[:],
                                     mybir.ActivationFunctionType.Ln,
                                     bias=epsb[:], scale=1.0 / HD)
                rs = sb.tile([1, TT], f32, tag="rs", name="rs")
                nc.scalar.activation(rs[:], lnms[:],
                                     mybir.ActivationFunctionType.Exp,
                                     bias=0.0, scale=-0.5)
                rsb = sb.tile([128, TT], f32, tag="rsb", name="rsb")
                nc.gpsimd.partition_broadcast(rsb[:], rs[:])
                # rotary, even dims on partitions 0:64, odd on 64:128:
                #   a      = q * fr            (both halves at once)
                #   bswap  = swap_halves(q) * [+fi; -fi]  (2 cross-half muls;
                #            the sign baked into fi makes the combine an add)
                #   rot    = a + bswap
                # kept fp32 until the final bf16 store: the rounding error
                # of bf16 intermediates is amplified ~|score| by the exp
                rot = sb.tile([128, TT], f32, tag="rot", name="rot")
                a = sb.tile([128, TT], f32, tag="rota", name="a")
                nc.vector.tensor_mul(a[:], ps[:], fr[:, ts])
                bsw = sb.tile([128, TT], f32, tag="rotb", name="bsw")
                nc.vector.tensor_mul(bsw[0:64, :], ps[64:128, :],
                                     fi[64:128, ts])
                nc.vector.tensor_mul(bsw[64:128, :], ps[0:64, :],
                                     fi[0:64, ts])
                nc.vector.tensor_add(rot[:], a[:], bsw[:])
                dst = krot if is_k else qrot[chunk]
                nc.vector.tensor_mul(dst[:, ts], rot[:], rsb[:])

        def project_chunk(chunk, wch=None):
            if wch is None:
                wch = load_wch(chunk)
            for tt in range(N_TT):
                ts = slice(tt * TT, (tt + 1) * TT)
                ps = pproj.tile([128, TT], f32, tag="proj", name="ps")
                for kt in range(N_KT):
                    nc.tensor.matmul(
                        ps[:], wch[:, kt, :],
                        xT[:, kt, ts], start=(kt == 0), stop=(kt == N_KT - 1))
                proj_consume(chunk, tt, ps)

        # ---------- emission: projections ----------
        # The k chunk runs while x is still streaming in: tiles are
        # interleaved with a phase lag so the lagging tiles' matmuls
        # (whose x pieces arrived long ago) fill the DMA arrival gaps
        # that otherwise idle the PE during the ramp, and the tiles
        # finish staggered so their rms/rotary consumer chains pipeline.
        # The half-1 ramp group adds the two half-0 v tiles (x resident,
        # weights preloaded) as always-ready filler.
        LAGK = 3
        KCH, VCH = HEADS_PER_CORE, HEADS_PER_CORE + 1

        def ramp_group(plan):
            # plan: list of (chunk, wch, tt, lag)
            pss, tss = [], []
            for chunk, wch, tt, lag in plan:
                pss.append(pproj.tile([128, TT], f32, tag="proj", name="ps"))
                tss.append(slice(tt * TT, (tt + 1) * TT))
            maxlag = max(lag for _, _, _, lag in plan)
            for r in range(N_KT + maxlag):
                for (chunk, wch, tt, lag), ps, ts in zip(plan, pss, tss):
                    kt = r - lag
                    if 0 <= kt < N_KT:
                        nc.tensor.matmul(
                            ps[:], wch[:, kt, :], xT[:, kt, ts],
                            start=(kt == 0), stop=(kt == N_KT - 1))
            for (chunk, wch, tt, lag), ps in zip(plan, pss):
                proj_consume(chunk, tt, ps)

        ramp_group([(KCH, wch_next, 0, 0), (KCH, wch_next, 1, 2),
                    (VCH, wch_v, 0, 3), (VCH, wch_v, 1, 4)])
        ramp_group([(KCH, wch_next, 2, 0), (KCH, wch_next, 3, 2),
                    (VCH, wch_v, 2, 3), (VCH, wch_v, 3, 4)])
        for h in range(HEADS_PER_CORE):
            project_chunk(h)
        # v -> [token, dv] layout; emitted last so these ready-to-run PE ops
        # fill the gap while the final q chunk's rotary drains the PSUM
        # banks the attention pools alias
        for i in range(N_SKT):
            tp = ptp.tile([128, 128], bf16, tag="tp", name="tp")
            nc.tensor.transpose(tp[:], vT[:, i * 128:(i + 1) * 128], ident[:])
            nc.vector.tensor_copy(vtok[:, i, :], tp[:])
        # wo stream goes last on the sync queue: it is only needed by the
        # first out-projection (~40us later) and must not steal HBM
        # bandwidth from the x/wqkv stream that gates phase 1.
        nc.sync.dma_start(
            woT[:], woT_d.rearrange("(eo p) d -> p eo d", p=128))
        p1_ctx.close()   # xT/wqkvT/fr/fi + projection PSUM no longer needed

        # ---------- phase 2 pools (reuse the PSUM banks phase 1 freed) ----
        psum = ctx.enter_context(tc.tile_pool(name="psum", bufs=2,
                                              space="PSUM"))
        pacc = ctx.enter_context(tc.tile_pool(name="pacc", bufs=1,
                                              space="PSUM"))
        pout = ctx.enter_context(tc.tile_pool(name="pout", bufs=2,
                                              space="PSUM"))

        # ---------- out-projection m-tile unit (PE filler work) ----------
        # evacuation alternates DVE/ACT so consecutive m-tiles' PSUM-bank
        # releases overlap (the dense tail is otherwise evacuation-paced)
        evac_flip = [0]

        def outproj_mtile(qt, m):
            qs = slice(qt * TT, (qt + 1) * TT)
            ops = pout.tile([128, TT], f32, tag="oproj", name="ops")
            for e in range(HEADS_PER_CORE):
                nc.tensor.matmul(ops[:],
                                 woT[:, e, m * 128:(m + 1) * 128],
                                 yT[e][:, qs], start=(e == 0),
                                 stop=(e == HEADS_PER_CORE - 1))
            osb = osbp.tile([128, TT], bf16, tag="osb", name="osb")
            if evac_flip[0] % 2:
                nc.scalar.copy(osb[:], ops[:])
            else:
                nc.vector.tensor_copy(osb[:], ops[:])
            evac_flip[0] += 1
            nc.sync.dma_start(outT_d[m * 128:(m + 1) * 128, qs], osb[:])

        pending = []     # out-proj m-tiles ready to weave into the PE stream

        def weave_one():
            if pending:
                outproj_mtile(*pending.pop(0))

        # ---------- attention unit (head h, query tile qt) ----------
        # Two key-tiles of scores share one 2-bank fp32 PSUM tile and one
        # exp instruction (halves the per-exp overhead), and the denom/y
        # consumer matmuls are software-pipelined LAG pairs behind the
        # score matmuls: the exp latency is then never on the PE critical
        # path, and yps/dps single-buffer without boundary stalls (a score
        # PSUM tile frees at its exp, not at its consumers).
        LAG = 2

        def attention_unit(h, qt, weave=0):
            ntk = 4 * (qt + 1)
            npair = ntk // 2
            dps = prow.tile([1, TT], f32, tag="row", name="dps")
            yps = pacc.tile([128, TT], f32, tag="yacc", name="yps")
            stages = []    # per-pair consumer args: (e, halves)

            def emit_scores(p):
                sps = psum.tile([128, 2, TT], f32, tag="mm", name="sps")
                halves = []
                for i in (0, 1):
                    # diagonal tiles (r >= 1) only have valid scores in
                    # their last TT - 128*r columns; skip the fully-masked
                    # prefix.  In suffix-local coords the causal mask is
                    # always the r=0 triangle.
                    tk = 2 * p + i
                    r = tk - 4 * qt
                    off = 128 * r if r > 0 else 0
                    qs = slice(qt * TT + off, (qt + 1) * TT)
                    nc.tensor.matmul(sps[:, i, off:],
                                     krot[:, tk * 128:(tk + 1) * 128],
                                     qrot[h][:, qs], start=True, stop=True)
                    halves.append((tk, r, off, TT - off))
                e = epool.tile([128, 2, TT], bf16, tag="e", name="e")
                full = halves[0][1] < 0 and halves[1][1] < 0
                if full:
                    nc.scalar.activation(e[:], sps[:],
                                         mybir.ActivationFunctionType.Exp,
                                         bias=0.0, scale=SCALE)
                else:
                    for i, (tk, r, off, w) in enumerate(halves):
                        nc.scalar.activation(
                            e[:, i, off:], sps[:, i, off:],
                            mybir.ActivationFunctionType.Exp,
                            bias=0.0, scale=SCALE)
                # DVE products are hoisted here (LAG pairs ahead of their
                # consumer matmuls) so the PE never waits on the exp->DVE
                # two-hop chain at the head of its queue
                srcs = []
                e01 = None
                if full:
                    # one denominator matmul per pair on the DVE pair-sum
                    # (single bf16 rounding of the pair, not a running sum)
                    e01 = epool.tile([128, TT], bf16, tag="em", name="e01")
                    nc.vector.tensor_add(e01[:], e[:, 0, :], e[:, 1, :])
                    srcs = [e[:, 0, :], e[:, 1, :]]
                else:
                    for i, (tk, r, off, w) in enumerate(halves):
                        if r >= 0:
                            em = epool.tile([128, TT], bf16, tag="em",
                                            name="em")
                            nc.vector.tensor_mul(em[:, :w], e[:, i, off:],
                                                 cmask[:, :w])
                            srcs.append(em[:, :w])
                        else:
                            srcs.append(e[:, i, off:])
                stages.append((halves, srcs, e01))

            def emit_consumers(p):
                halves, srcs, e01 = stages[p]
                for i, (tk, r, off, w) in enumerate(halves):
                    nc.tensor.matmul(yps[:, off:], vtok[:, tk, :], srcs[i],
                                     start=(tk == 0), stop=(tk == ntk - 1))
                if e01 is not None:
                    nc.tensor.matmul(dps[:], ones[:], e01[:],
                                     start=(halves[0][0] == 0),
                                     stop=(halves[1][0] == ntk - 1))
                else:
                    for i, (tk, r, off, w) in enumerate(halves):
                        nc.tensor.matmul(dps[:, off:], ones[:], srcs[i],
                                         start=(tk == 0),
                                         stop=(tk == ntk - 1))

            for p in range(npair):
                emit_scores(p)
                if p >= LAG:
                    emit_consumers(p - LAG)
                # drop an always-ready out-proj unit into the PE queue so
                # the engine has work while the next exp drains
                for _ in range(weave):
                    weave_one()
            for p in range(max(0, npair - LAG), npair):
                emit_consumers(p)
                # filler between the flushed consumers: their exps are the
                # freshest and otherwise stall the PE at unit boundaries
                if weave:
                    weave_one()
            qs = slice(qt * TT, (qt + 1) * TT)
            dr = sb.tile([1, TT], f32, tag="dr", name="dr")
            nc.vector.reciprocal_approx_fast(dr[:], dps[:])
            drb = sb.tile([128, TT], f32, tag="drb", name="drb")
            nc.gpsimd.partition_broadcast(drb[:], dr[:])
            nc.vector.tensor_mul(yT[h][:, qs], yps[:], drb[:])

        # ---------- emission: attention qt=3..0 with woven out-proj -------
        # out-proj for query tile qt becomes available once all 4 heads of
        # qt are done; it is woven into the following qt's attention.
        for qt in (3, 2, 1, 0):
            # more weave slots as attention units shrink
            weave = {3: 0, 2: 1, 1: 1, 0: 1}[qt]
            for h in range(HEADS_PER_CORE):
                attention_unit(h, qt, weave=weave)
            pending.extend((qt, m) for m in range(D // 128))
        while pending:
            weave_one()

    nc.compile()
    return nc


def _host_shards(x, freqs_cis, wqkv, wo):
    import ml_dtypes
    bf16 = ml_dtypes.bfloat16

    # head-dim permutation: even dims then odd dims (for q and k only)
    perm = np.concatenate([np.arange(0, HD, 2), np.arange(1, HD, 2)])

    wq = wqkv[:Q_SIZE].reshape(NH, HD, D)[:, perm, :]
    wk = wqkv[Q_SIZE:Q_SIZE + KV_SIZE].reshape(NKV, HD, D)[:, perm, :]
    wv = wqkv[Q_SIZE + KV_SIZE:].reshape(NKV, HD, D)

    fr1 = np.ascontiguousarray(freqs_cis[:, :, 0].T, dtype=np.float32)
    fi1 = np.ascontiguousarray(freqs_cis[:, :, 1].T, dtype=np.float32)
    fr = np.vstack([fr1, fr1])
    # sign baked in so the rotary combine is a single add:
    #   rot[lo] = q_lo*fr + q_hi*(-fi) ; rot[hi] = q_hi*fr + q_lo*(+fi)
    fi = np.vstack([fi1, -fi1])

    # causal mask for the leading diagonal of a 128-row x 512-col score
    # tile (suffix-narrowed diagonal tiles reuse its prefix columns)
    tkl = np.arange(128)[:, None]
    tql = np.arange(TT)[None, :]
    mask = (tkl <= tql).astype(bf16)

    in_maps = []
    for c in range(N_CORES):
        b, j = divmod(c, TPC)
        wshard = np.concatenate(
            [wq[TPC * j + h] for h in range(HEADS_PER_CORE)] +
            [wk[j], wv[j]], axis=0)                     # (768, D)
        # [chunk, p, ko, e] with d = ko*128 + p
        wpack = np.ascontiguousarray(
            wshard.reshape(HEADS_PER_CORE + 2, HD, N_KT, 128)
            .transpose(0, 3, 2, 1)).astype(bf16)
        in_maps.append({
            "xT": np.ascontiguousarray(x[b].T).astype(bf16),
            "wqkvT": wpack,
            "woT": np.ascontiguousarray(
                wo[:, j * E_LOC:(j + 1) * E_LOC].T).astype(bf16),
            "fr": fr,
            "fi": fi,
            "mask": mask,
        })
    return in_maps


_NC_CACHE = {}


def _get_nc():
    if "nc" not in _NC_CACHE:
        _NC_CACHE["nc"] = _build_bass()
    return _NC_CACHE["nc"]


def kernel(x, freqs_cis, wqkv, wo, q_norm_w, k_norm_w, _want_results=False):
    # q_norm_w / k_norm_w are all-ones per the problem spec; rmsnorm weight
    # multiply is the identity and is folded away.
    from concourse.bass_utils import run_bass_kernel_spmd

    nc = _get_nc()
    in_maps = _host_shards(np.asarray(x, np.float32),
                           np.asarray(freqs_cis, np.float32),
                           np.asarray(wqkv, np.float32),
                           np.asarray(wo, np.float32))
    res = run_bass_kernel_spmd(nc, in_maps, core_ids=list(range(N_CORES)))
    parts = [r["outT"] for r in res.results]
    out = np.empty((B, S, D), np.float32)
    for b in range(B):
        acc = parts[TPC * b].astype(np.float32)
        for j in range(1, TPC):
            acc = acc + parts[TPC * b + j].astype(np.float32)
        out[b] = acc.T
    if _want_results:
        return out, res
    return out
